# revision 23
# baseline (speedup 1.0000x reference)
"""Swin-style windowed multi-head attention on 8 Trainium2 NeuronCores.

Problem: nn_Attention_86792699118108
  x [16, 3136, 768], 56x56 spatial, window 14x14 (no padding needed),
  12 heads, head_dim 64. 256 independent windows -> 32 windows per core.

Strategy (data-parallel over windows):
  host: window-partition x, cast to bf16, pre-transpose/scale weights,
        pre-gather the relative-position bias table (static indices),
        exponentiate it (exp(s + b) == exp(s) * exp(b)) and permute its
        head order to match the on-chip psum packing.
  device (per core, SPMD), per group of 4 windows, software-pipelined:
    xT (chan-major) via DMA-transpose ->
    qkv GEMM (q,k head-dim-major; v token-major with interleaved ones col)
    emitted as fine-grained "micro" thunks interleaved into the PREVIOUS
    group's attention windows so the PE never idles ->
    per window: scores_T = k.T q per head (PE, 4 heads packed per 2-bank
    psum tile, row-tile pairs run concurrently) -> exp (ACT, one op per
    4-head tile; no max-subtraction: scores are provably small) ->
    * exp(rpb) (DVE) -> [proj of the PREVIOUS window fills the softmax
    latency gap] -> AV: out_T[d, n] with ones row giving softmax sums for
    free (4 heads per psum tile) -> reciprocal (DVE, direct from psum) ->
    gpsimd partition-broadcast -> normalize on eviction (DVE) ->
    proj GEMM deferred one window -> token-major f32 out -> DMA.
  host: window-reverse.

  PSUM: 3 rotating 2-bank slots (scores/AV/proj) + 1 slot for qkv thunk
  accumulation = all 8 banks.  PSUM evictions run on the scalar engine
  (Copy activation) in the zero-bias fast path to keep DVE under the PE
  roofline; a general path applies biases on DVE.
"""

import numpy as np
import ml_dtypes

WS = 14
NH = 12
HD = 64
C = 768
N = WS * WS  # 196 tokens per window
NCORES = 8

_BF16 = ml_dtypes.bfloat16

_prog_cache = {}

# head order inside a 4-head psum/ex block: column-ascending order is
# s' = (0, 2, 1, 3) (even heads -> PE rows 0:64, odd heads -> rows 64:128;
# concurrent row-tile pairs must land in different psum banks).
_SPERM = (0, 2, 1, 3)
# scores psum column offset per within-block head index s'
_SCOL = (0, 512, 196, 708)
# AV psum column offset per within-block head index s' (ascending)
_ACOL = (0, 196, 512, 708)
# global head permutation: ex/attn/er column block b holds head PERM[b]
PERM = [4 * a + _SPERM[b] for a in range(3) for b in range(4)]


def _rel_index(ws):
    coords = np.stack(np.meshgrid(np.arange(ws), np.arange(ws), indexing="ij"))
    cf = coords.reshape(2, -1)
    rel = (cf[:, :, None] - cf[:, None, :]).transpose(1, 2, 0).astype(np.int64)
    rel[..., 0] += ws - 1
    rel[..., 1] += ws - 1
    rel[..., 0] *= 2 * ws - 1
    return rel.sum(-1)


def _build_program(n_win, has_bias):
    import concourse.bass as bass
    import concourse.mybir as mybir
    import concourse.tile as tile
    from concourse import bacc
    from contextlib import ExitStack

    assert n_win % 4 == 0
    n_grp = n_win // 4
    n_tok = n_win * N

    BF = mybir.dt.bfloat16
    F32 = mybir.dt.float32
    AF = mybir.ActivationFunctionType

    MC = [(0, 128), (128, 68)]  # token/key chunks within a 196-token window

    nc = bacc.Bacc("TRN2", target_bir_lowering=False, debug=False,
                   num_devices=NCORES)

    x = nc.dram_tensor("x", [n_tok, C], BF, kind="ExternalInput")
    wqkvT = nc.dram_tensor("wqkvT", [C, 3 * C], BF, kind="ExternalInput")
    wpT = nc.dram_tensor("wpT", [C, C], BF, kind="ExternalInput")
    er = nc.dram_tensor("er", [N, NH * N], BF, kind="ExternalInput")
    qkb = nc.dram_tensor("qkb", [128, 12], F32, kind="ExternalInput")
    vb = nc.dram_tensor("vb", [1, C], F32, kind="ExternalInput")
    pb = nc.dram_tensor("pb", [1, C], F32, kind="ExternalInput")
    y = nc.dram_tensor("y", [n_tok, C], F32, kind="ExternalOutput")

    def bcast_ap(handle, p):
        a = handle[:, :]
        return bass.AP(tensor=a.tensor, offset=a.offset, ap=[[0, p], [1, C]])

    def sview(t_ap, dims):
        """Strided multi-dim view of a 2D tile AP: dims = [[stride, num] ...]
        (first entry is the partition dim, stride in partitions)."""
        return bass.AP(tensor=t_ap.tensor, offset=t_ap.offset, ap=list(dims))

    with ExitStack() as ctx:
        tc = ctx.enter_context(tile.TileContext(nc))
        consts = ctx.enter_context(tc.tile_pool(name="consts", bufs=1))
        grp = ctx.enter_context(tc.tile_pool(name="grp", bufs=2))
        win = ctx.enter_context(tc.tile_pool(name="win", bufs=2))
        # PSUM: tag "ps" = 3 rotating 2-bank slots (scores / AV / proj),
        # tag "pt" = 1 slot for the interleaved qkv-thunk accumulations.
        # NOTE: only ONE matmul accumulation group per psum BANK at a time;
        # concurrent row-tile score pairs are placed in different banks.
        psp = ctx.enter_context(tc.tile_pool(name="psp", bufs=1, space="PSUM"))

        def emit_xT(g):
            t0 = g * 4 * N
            xT = []
            for ic in range(6):
                t = grp.tile([128, 4 * N], BF, tag=f"xT{ic}", name=f"xT{ic}")
                nc.sync.dma_start(
                    out=t,
                    in_=x[t0:t0 + 4 * N, ic * 128:(ic + 1) * 128],
                    transpose=True)
                xT.append(t)
            return xT

        # ---- constants (ordered so the first qk thunks start ASAP) -----
        wq = [consts.tile([128, 3 * C], BF, tag=f"wq{ic}", name=f"wq{ic}")
              for ic in range(6)]
        nc.sync.dma_start(out=wq[0], in_=wqkvT[0:128, :])
        xT0 = emit_xT(0)
        for ic in range(1, 6):
            nc.sync.dma_start(out=wq[ic], in_=wqkvT[ic * 128:(ic + 1) * 128, :])
        wp = []
        for ic in range(6):
            t = consts.tile([128, C], BF, tag=f"wp{ic}", name=f"wp{ic}")
            nc.sync.dma_start(out=t, in_=wpT[ic * 128:(ic + 1) * 128, :])
            wp.append(t)
        er_t = []
        for mci, (mo, msz) in enumerate(MC):
            t = consts.tile([msz, NH * N], BF, tag=f"er{mci}", name=f"er{mci}")
            nc.sync.dma_start(out=t, in_=er[mo:mo + msz, :])
            er_t.append(t)
        qkb_t = consts.tile([128, 12], F32, tag="qkb", name="qkb_t")
        nc.sync.dma_start(out=qkb_t, in_=qkb[:, :])
        vb_t = consts.tile([128, C], F32, tag="vb", name="vb_t")
        nc.gpsimd.dma_start(out=vb_t, in_=bcast_ap(vb, 128))
        pb_t = consts.tile([128, C], F32, tag="pb", name="pb_t")
        nc.gpsimd.dma_start(out=pb_t, in_=bcast_ap(pb, 128))

        def make_thunks(xT, ps_tag):
            """qkv emission for one group as 20 thunks x 3 micro-steps of
            4 matmuls, so they can be interleaved into the previous group's
            attention windows (per-engine streams are strictly in-order;
            this puts filler PE work inside the softmax chain-latency
            gaps). Each thunk's psum eviction is emitted with its last
            micro."""
            qk = [grp.tile([128, 4 * N], BF, tag=f"qk{oc}", name=f"qk{oc}")
                  for oc in range(12)]
            v_t = {}
            thunks = []

            def mk_qk(oc):
                st = {}

                def mm(step):
                    # 12 MMs as two SEQUENTIAL 6-ic accumulation groups
                    # (walrus allows one open group per psum tile).
                    if step == 0:
                        st["ps"] = psp.tile([128, 1024], F32, tag=ps_tag,
                                            name="psqkv", bufs=(3 if ps_tag == "ps" else 1))
                    ps = st["ps"]
                    for k in range(step * 4, step * 4 + 4):
                        s, ic = divmod(k, 6)
                        nc.tensor.matmul(
                            ps[:, s * 512:s * 512 + 392],
                            wq[ic][:, oc * 128:(oc + 1) * 128],
                            xT[ic][:, s * 392:(s + 1) * 392],
                            start=(ic == 0), stop=(ic == 5))

                def evict():
                    ps = st["ps"]
                    pv = ps.rearrange("p (s n) -> p s n", s=2)[:, :, 0:392]
                    ov = qk[oc][:, :].rearrange("p (s n) -> p s n", s=2)
                    if has_bias:
                        nc.vector.tensor_scalar_add(ov, pv, qkb_t[:, oc:oc + 1])
                    else:
                        nc.scalar.activation(ov, pv, AF.Copy)

                return [lambda i=i: mm(i) for i in range(3)], evict

            for oc in range(12):
                thunks.append(mk_qk(oc))

            def mk_v(w4, mci):
                mo, msz = MC[mci]
                vt = grp.tile([128, NH * 65], BF,
                              tag=f"v{w4}_{mci}", name=f"v{w4}_{mci}")
                v_t[(w4, mci)] = vt
                st = {}

                def mm(step):
                    if step == 0:
                        st["ps"] = psp.tile([128, 1024], F32, tag=ps_tag,
                                            name="psv", bufs=(3 if ps_tag == "ps" else 1))
                    ps = st["ps"]
                    for k in range(step * 4, step * 4 + 4):
                        half, ic = divmod(k, 6)
                        nc.tensor.matmul(
                            ps[:msz, half * 512:half * 512 + 384],
                            xT[ic][:, w4 * N + mo: w4 * N + mo + msz],
                            wq[ic][:, 1536 + half * 384: 1536 + (half + 1) * 384],
                            start=(ic == 0), stop=(ic == 5))

                def evict():
                    ps = st["ps"]
                    vr = vt.rearrange("p (h e) -> p h e", e=65)
                    pv = (ps.rearrange("p (s j e) -> p s j e", s=2, e=64)
                          [:msz, :, 0:6, :])
                    ov = (vt.rearrange("p (s j e) -> p s j e", s=2, e=65)
                          [:msz, :, :, 0:64])
                    if has_bias:
                        bv = (vb_t.rearrange("p (s j e) -> p s j e", s=2, e=64)
                              [:msz])
                        nc.vector.tensor_add(ov, pv, bv)
                    else:
                        nc.vector.tensor_copy(ov, pv)
                    nc.vector.memset(vr[:msz, :, 64:65], 1.0)

                return [lambda i=i: mm(i) for i in range(3)], evict

            for w4 in range(4):
                for mci in range(2):
                    thunks.append(mk_v(w4, mci))
            return qk, v_t, thunks

        class Queue:
            def __init__(self, thunks):
                self.items = []
                for micros, evict in thunks:
                    for k, m in enumerate(micros):
                        if k == len(micros) - 1:
                            self.items.append((m, evict))
                        else:
                            self.items.append((m, None))
                self.i = 0

            def pop(self):
                if self.i < len(self.items):
                    m, ev = self.items[self.i]
                    self.i += 1
                    m()
                    if ev is not None:
                        ev()

            def drain(self):
                while self.i < len(self.items):
                    self.pop()

        def emit_proj(prev):
            """proj GEMM + eviction + y DMA for a previously finished
            window (aoT tiles captured in prev)."""
            aoT, t0, w0 = prev
            for mci, (mo, msz) in enumerate(MC):
                pp = psp.tile([128, 1024], F32, tag="ps", name="psproj", bufs=3)
                for half in range(2):
                    for ic in range(6):
                        nc.tensor.matmul(
                            pp[:msz, half * 512:half * 512 + 384],
                            aoT[ic][:, mo:mo + msz],
                            wp[ic][:, half * 384:(half + 1) * 384],
                            start=(ic == 0), stop=(ic == 5))
                ysb = win.tile([128, C], F32, tag="ysb", name="ysb", bufs=3)
                pv = pp.rearrange("p (s n) -> p s n", s=2)[:msz, :, 0:384]
                ov = ysb[:msz, :].rearrange("p (s n) -> p s n", s=2)
                if has_bias:
                    bv = pb_t[:msz, :].rearrange("p (s n) -> p s n", s=2)
                    nc.vector.tensor_add(ov, pv, bv)
                else:
                    nc.vector.tensor_copy(ov, pv)
                nc.sync.dma_start(
                    out=y[t0 + w0 + mo: t0 + w0 + mo + msz, :],
                    in_=ysb[:msz, :])

        def emit_window(g, w4, qk, v_t, queue, prev):
            """One window: scores -> exp -> *er -> [proj of prev window]
            -> AV -> recip/broadcast/normalize.  Returns proj context."""
            t0 = g * 4 * N
            w0 = w4 * N

            ex = [win.tile([msz, NH * N], BF, tag=f"ex{mci}", name=f"ex{mci}")
                  for mci, (mo, msz) in enumerate(MC)]
            attn = [win.tile([msz, NH * N], BF, tag=f"at{mci}", name=f"at{mci}")
                    for mci, (mo, msz) in enumerate(MC)]

            # ---- scores + exp + er-mul, filler interleaved -------------
            # er-mul is split per 4-head block so AV tile a can start as
            # soon as its own blocks are through exp (both mci).
            for mci, (mo, msz) in enumerate(MC):
                for a in range(3):
                    ps = psp.tile([128, 1024], F32, tag="ps", name="psqk", bufs=3)
                    for sp in range(4):
                        h = 4 * a + sp
                        i = h // 2
                        ro = (h % 2) * 64
                        nc.tensor.matmul(
                            ps[:msz, _SCOL[sp]:_SCOL[sp] + N],
                            qk[6 + i][ro:ro + 64, w0 + mo: w0 + mo + msz],
                            qk[i][ro:ro + 64, w0:w0 + N],
                            start=True, stop=True)
                    queue.pop()
                    if mci == 0 and a == 0:
                        queue.pop()
                    pv = ps.rearrange("p (b n) -> p b n", b=2)[:msz, :, 0:2 * N]
                    ov = (ex[mci][:, a * 4 * N:(a + 1) * 4 * N]
                          .rearrange("p (b n) -> p b n", b=2))
                    nc.scalar.activation(ov, pv, AF.Exp)
                    nc.vector.tensor_mul(
                        attn[mci][:, a * 4 * N:(a + 1) * 4 * N],
                        ex[mci][:, a * 4 * N:(a + 1) * 4 * N],
                        er_t[mci][:, a * 4 * N:(a + 1) * 4 * N])
            for _ in range(3):
                queue.pop()

            # ---- previous window's proj fills the softmax latency gap --
            if prev is not None:
                emit_proj(prev)

            # ---- AV + normalize ----------------------------------------
            aoT = [win.tile([128, N], BF, tag=f"aoT{i}", name=f"aoT{i}")
                   for i in range(6)]
            for a in range(3):
                queue.pop()
                pa = psp.tile([65, 1024], F32, tag="ps", name="psav", bufs=3)
                for sp in range(4):
                    h = 4 * a + sp
                    blk = PERM[h]
                    for mci, (mo, msz) in enumerate(MC):
                        nc.tensor.matmul(
                            pa[:, _ACOL[sp]:_ACOL[sp] + N],
                            v_t[(w4, mci)][:msz, h * 65:(h + 1) * 65],
                            attn[mci][:, blk * N:(blk + 1) * N],
                            start=(mci == 0), stop=(mci == 1))
                # Evict the AV tile via two ACT copies (sums row shifted to
                # partition 0 — custom-DVE/gpsimd ops need base partition 0;
                # dims rows unshifted) — releases the psum slot ~2us after
                # the MMs so the next window's scores aren't gated on the
                # normalize chain, which then runs entirely from SBUF.
                sm = win.tile([1, 4 * N], F32, tag="sm", name="sm", bufs=2)
                nc.scalar.activation(
                    sm[:, :].rearrange("p (b n) -> p b n", b=2),
                    pa.rearrange("p (b n) -> p b n", b=2)[64:65, :, 0:2 * N],
                    AF.Copy)
                aod = win.tile([64, 4 * N], F32, tag="aod", name="aod", bufs=2)
                nc.scalar.activation(
                    aod[:, :].rearrange("p (b n) -> p b n", b=2),
                    pa.rearrange("p (b n) -> p b n", b=2)[0:64, :, 0:2 * N],
                    AF.Copy)
                rr = win.tile([1, 4 * N], F32, tag="rr", name="rr", bufs=2)
                nc.vector.reciprocal_approx_fast(rr, sm)
                rrep = win.tile([64, 4 * N], F32, tag="rrep", name="rrep",
                                bufs=2)
                nc.gpsimd.partition_broadcast(rrep, rr)
                for sp in range(4):
                    h = 4 * a + sp
                    i = h // 2
                    ro = (h % 2) * 64
                    nc.vector.tensor_mul(
                        aoT[i][ro:ro + 64, :],
                        aod[0:64, sp * N:(sp + 1) * N],
                        rrep[0:64, sp * N:(sp + 1) * N])
            for _ in range(2):
                queue.pop()

            return (aoT, t0, w0)

        # ---- software pipeline over groups -----------------------------
        qk_c, vt_c, th0 = make_thunks(xT0, "ps")
        for micros, evict in th0:
            for m in micros:
                m()
            evict()

        prev = None
        for g in range(n_grp):
            if g + 1 < n_grp:
                xTn = emit_xT(g + 1)
                qk_n, vt_n, thunks = make_thunks(xTn, "pt")
            else:
                qk_n, vt_n, thunks = None, None, []
            queue = Queue(thunks)
            for w4 in range(4):
                prev = emit_window(g, w4, qk_c, vt_c, queue, prev)
            queue.drain()
            qk_c, vt_c = qk_n, vt_n
        emit_proj(prev)

    nc.compile()
    return nc


def _get_program(n_win, has_bias=False):
    key = (n_win, has_bias)
    if key not in _prog_cache:
        _prog_cache[key] = _build_program(n_win, has_bias)
    return _prog_cache[key]


def _host_prep(x, qkv_w, q_bias, v_bias, rel_bias_table, proj_w, proj_b, H, W):
    B = x.shape[0]
    nws = H // WS  # windows per side
    xw = (np.asarray(x, np.float32)
          .reshape(B, nws, WS, nws, WS, C)
          .transpose(0, 1, 3, 2, 4, 5)
          .reshape(-1, N, C))  # [Bw, 196, C]

    scale = HD ** -0.5
    wq_s = np.array(qkv_w, np.float32, copy=True)
    wq_s[0:C] *= scale
    wqkvT = np.ascontiguousarray(wq_s.T).astype(_BF16)
    wpT = np.ascontiguousarray(np.asarray(proj_w, np.float32).T).astype(_BF16)

    idx = _rel_index(WS).reshape(-1)
    rpb = np.asarray(rel_bias_table, np.float32)[idx].reshape(N, N, NH)  # [n,m,h]
    # [m, h, n], head order permuted to match on-chip psum packing
    rpb_t = np.exp(rpb).transpose(1, 2, 0)
    er = np.ascontiguousarray(
        rpb_t[:, PERM, :].reshape(N, NH * N)).astype(_BF16)

    qkv_b = np.concatenate([
        np.asarray(q_bias, np.float32) * scale,
        np.zeros(C, np.float32),
        np.asarray(v_bias, np.float32)])
    qkb = np.ascontiguousarray(qkv_b[0:2 * C].reshape(12, 128).T)
    vbias = np.ascontiguousarray(np.asarray(v_bias, np.float32).reshape(1, C))
    pbias = np.ascontiguousarray(np.asarray(proj_b, np.float32).reshape(1, C))

    xbf = np.ascontiguousarray(xw.reshape(-1, C)).astype(_BF16)
    return xbf, wqkvT, wpT, er, qkb, vbias, pbias


def kernel(x, qkv_w, q_bias, v_bias, rel_bias_table, proj_w, proj_b, H, W,
           _return_results=False):
    from concourse.bass_utils import run_bass_kernel_spmd

    x = np.asarray(x)
    B = x.shape[0]
    H = int(H)
    W = int(W)
    nws = H // WS

    xbf, wqkvT, wpT, er, qkb, vbias, pbias = _host_prep(
        x, qkv_w, q_bias, v_bias, rel_bias_table, proj_w, proj_b, H, W)

    has_bias = not (np.all(np.asarray(q_bias) == 0)
                    and np.all(np.asarray(v_bias) == 0)
                    and np.all(np.asarray(proj_b) == 0))

    Bw = B * nws * nws
    n_win_core = Bw // NCORES
    nc = _get_program(n_win_core, has_bias)

    tok_core = n_win_core * N
    in_maps = []
    for c in range(NCORES):
        in_maps.append({
            "x": xbf[c * tok_core:(c + 1) * tok_core],
            "wqkvT": wqkvT, "wpT": wpT, "er": er,
            "qkb": qkb, "vb": vbias, "pb": pbias,
        })

    res = run_bass_kernel_spmd(nc, in_maps, list(range(NCORES)))
    yw = np.concatenate([res.results[c]["y"] for c in range(NCORES)], axis=0)
    out = (yw.reshape(B, nws, nws, WS, WS, C)
           .transpose(0, 1, 3, 2, 4, 5)
           .reshape(B, H * W, C).astype(np.float32))
    if _return_results:
        return out, res
    return out
